# revision 16
# baseline (speedup 1.0000x reference)
"""Trainium2 Bass kernel for nn_DriftingPolicy (Nadaraya-Watson RBF drift field).

Computes v = -drift(x, y_pos) + 0.5*drift(x, y_neg) where
  drift(x, y)_i = x_i * (s_i/denom_i) - (w @ y)_i / denom_i
  w_ij = exp(-||x_i - y_j||^2 / 2), diagonal (i==j) masked, s = rowsum(w),
  denom = max(s, 1e-8).

Sharding: rows of x (B=4096) split across 8 cores (512 rows each); y_pos/y_neg
replicated.  Per core, flash-style loop over 32 j-tiles of y:
  dist:  dot[j,i]  = sum_d y[j,d] x[i,d]        (PE, lhsT = y.T tile)
  w_raw = exp(dot - 0.5*||y_j||^2)              (ACT, per-partition bias)
  accT[d,i] += sum_j y[j,d] w_raw[j,i]          (PE, accumulating)
  s_raw[i]  += sum_j w_raw[j,i]                 (PE, ones-vector lhsT)
The per-i factor exp(-0.5*||x_i||^2) and the diagonal-mask correction
(subtract w_ii, computed directly from x and the core's diagonal y rows)
are folded into the epilogue scalars.  Host pre-transposes x and y so no
on-device data transposes are needed in the main loop.
"""

import numpy as np

B, TA, DA = 4096, 16, 8
D = TA * DA            # 128
NCORES = 8
IW = B // NCORES       # 512 query rows per core
P = 128                # partitions
NT = B // P            # 32 j-tiles
NCH = IW // P          # 4 i-chunks per core
EPS = 1e-8

_CACHE = {}


def _build(repeat=1):
    import concourse.bass as bass
    import concourse.tile as tile
    from concourse import mybir
    from concourse.masks import make_identity
    from concourse.bass import ts
    from contextlib import ExitStack

    F32 = mybir.dt.float32
    Alu = mybir.AluOpType
    Act = mybir.ActivationFunctionType

    nc = bass.Bass()
    x_d = nc.declare_dram_parameter("x", [IW, D], F32, isOutput=False)
    F16 = mybir.dt.float16
    xTh_d = nc.declare_dram_parameter("xTh", [D, IW], F16, isOutput=False)
    xTl_d = nc.declare_dram_parameter("xTl", [D, IW], F16, isOutput=False)
    y_d = [
        nc.declare_dram_parameter("y_pos", [B, D], F32, isOutput=False),
        nc.declare_dram_parameter("y_neg", [B, D], F32, isOutput=False),
    ]
    yTh_d = [
        nc.declare_dram_parameter("yTh_pos", [D, B], F16, isOutput=False),
        nc.declare_dram_parameter("yTh_neg", [D, B], F16, isOutput=False),
    ]
    yTl_d = [
        nc.declare_dram_parameter("yTl_pos", [D, B], F16, isOutput=False),
        nc.declare_dram_parameter("yTl_neg", [D, B], F16, isOutput=False),
    ]
    yd_d = [
        nc.declare_dram_parameter("yd_pos", [IW, D], F32, isOutput=False),
        nc.declare_dram_parameter("yd_neg", [IW, D], F32, isOutput=False),
    ]
    ysq_d = [
        nc.declare_dram_parameter("ysqh_pos", [P, NT], F32, isOutput=False),
        nc.declare_dram_parameter("ysqh_neg", [P, NT], F32, isOutput=False),
    ]
    out_d = nc.declare_dram_parameter("out", [IW, D], F32, isOutput=True)

    with tile.TileContext(nc) as tc, ExitStack() as ctx:
        singles = ctx.enter_context(tc.tile_pool(name="singles", bufs=1))
        wpool = ctx.enter_context(tc.tile_pool(name="wpool", bufs=5))
        scrpool = ctx.enter_context(tc.tile_pool(name="scr", bufs=2))
        ps_dot = ctx.enter_context(tc.tile_pool(name="ps_dot", bufs=4, space="PSUM"))
        ps_acc = ctx.enter_context(tc.tile_pool(name="ps_acc", bufs=2, space="PSUM"))
        ps_s = ctx.enter_context(tc.tile_pool(name="ps_s", bufs=2, space="PSUM"))
        epi = ctx.enter_context(tc.tile_pool(name="epi", bufs=2))

        # ---- constants & inputs resident in SBUF ----
        ident = singles.tile([P, P], F32, name="ident", tag="ident")
        make_identity(nc, ident[:, :])
        ones = singles.tile([P, 1], F32, name="ones", tag="ones")
        nc.gpsimd.memset(ones[:, :], 1.0)

        HEAD = 4
        # tiles, allocated up front
        x_sb = singles.tile([P, NCH, D], F32, name="x_sb", tag="x_sb")
        xTh_sb = singles.tile([D, IW], F16, name="xTh_sb", tag="xTh_sb")
        xTl_sb = singles.tile([D, IW], F16, name="xTl_sb", tag="xTl_sb")
        yd_sb = [
            singles.tile([P, NCH, D], F32, name=f"yd{f}", tag=f"yd{f}")
            for f in range(2)
        ]
        y_sb = [
            singles.tile([P, NT, D], F32, name=f"y{f}", tag=f"y{f}")
            for f in range(2)
        ]
        yTh_sb = [
            singles.tile([D, B], F16, name=f"yTh{f}", tag=f"yTh{f}")
            for f in range(2)
        ]
        yTl_sb = [
            singles.tile([D, B], F16, name=f"yTl{f}", tag=f"yTl{f}")
            for f in range(2)
        ]
        ysq_sb = [
            singles.tile([P, NT], F32, name=f"ysq{f}", tag=f"ysq{f}")
            for f in range(2)
        ]
        y_ap = [y_d[f][:, :].rearrange("(t p) d -> p t d", p=P) for f in range(2)]
        # issue order == SP execution order: hot path (first tiles of field 0)
        # first, then bulk, then field 1, then epilogue-only data.
        nc.sync.dma_start(xTh_sb[:, :], xTh_d[:, :])
        nc.sync.dma_start(xTl_sb[:, :], xTl_d[:, :])
        nc.sync.dma_start(yTh_sb[0][:, 0 : HEAD * P], yTh_d[0][:, 0 : HEAD * P])
        nc.sync.dma_start(yTl_sb[0][:, 0 : HEAD * P], yTl_d[0][:, 0 : HEAD * P])
        nc.sync.dma_start(ysq_sb[0][:, :], ysq_d[0][:, :])
        nc.sync.dma_start(y_sb[0][:, 0:HEAD, :], y_ap[0][:, 0:HEAD, :])
        nc.sync.dma_start(yTh_sb[0][:, HEAD * P : B], yTh_d[0][:, HEAD * P : B])
        nc.sync.dma_start(yTl_sb[0][:, HEAD * P : B], yTl_d[0][:, HEAD * P : B])
        nc.sync.dma_start(y_sb[0][:, HEAD:NT, :], y_ap[0][:, HEAD:NT, :])
        nc.sync.dma_start(yTh_sb[1][:, :], yTh_d[1][:, :])
        nc.sync.dma_start(yTl_sb[1][:, :], yTl_d[1][:, :])
        nc.sync.dma_start(ysq_sb[1][:, :], ysq_d[1][:, :])
        nc.sync.dma_start(y_sb[1][:, :, :], y_ap[1][:, :, :])
        nc.sync.dma_start(x_sb[:, :, :], x_d[:, :].rearrange("(c p) d -> p c d", p=P))
        for f in range(2):
            nc.sync.dma_start(
                yd_sb[f][:, :, :],
                yd_d[f][:, :].rearrange("(c p) d -> p c d", p=P),
            )

        # ---- per-row scalars: xsqh = -0.5*||x_i||^2, exb = exp(xsqh),
        #      wii_f = exp(-0.5*||x_i - ydiag_i||^2) ----
        xsq = singles.tile([P, NCH], F32, name="xsq", tag="xsq")
        for ch in range(NCH):
            scr = scrpool.tile([P, D], F32, name="scr", tag="scr")
            nc.vector.tensor_mul(scr[:, :], x_sb[:, ch, :], x_sb[:, ch, :])
            nc.vector.reduce_sum(
                xsq[:, ch : ch + 1], scr[:, :], axis=mybir.AxisListType.X
            )
        exb = singles.tile([P, NCH], F32, name="exb", tag="exb")
        nc.scalar.activation(exb[:, :], xsq[:, :], Act.Exp, scale=-0.5)

        wii = []
        for f in range(2):
            d2 = singles.tile([P, NCH], F32, name=f"d2_{f}", tag=f"d2_{f}")
            for ch in range(NCH):
                diff = scrpool.tile([P, D], F32, name="diff", tag="scr")
                nc.vector.tensor_sub(diff[:, :], x_sb[:, ch, :], yd_sb[f][:, ch, :])
                scr2 = scrpool.tile([P, D], F32, name="scr2", tag="scr")
                nc.vector.tensor_mul(scr2[:, :], diff[:, :], diff[:, :])
                nc.vector.reduce_sum(
                    d2[:, ch : ch + 1], scr2[:, :], axis=mybir.AxisListType.X
                )
            w = singles.tile([P, NCH], F32, name=f"wii{f}", tag=f"wii{f}")
            nc.scalar.activation(w[:, :], d2[:, :], Act.Exp, scale=-0.5)
            wii.append(w)

        # ---- main loop: two fields, 32 j-tiles each ----
        accT_sb = []   # [d, i] accumulators copied to SBUF
        srows = [
            singles.tile([1, IW], F32, name="srow0", tag="srow0"),
            singles.tile([1, IW], F32, name="srow1", tag="srow1"),
        ]
        def emit_dist(f, t):
            dot_ps = ps_dot.tile([P, IW], F32, name="dot_ps", tag="dot")
            # split-fp16 fp32 emulation: yh*xh + yh*xl + yl*xh  (ll term ~1e-6)
            nc.tensor.matmul(
                dot_ps[:, :], lhsT=yTh_sb[f][:, ts(t, P)], rhs=xTh_sb[:, :],
                start=True, stop=False,
            )
            nc.tensor.matmul(
                dot_ps[:, :], lhsT=yTh_sb[f][:, ts(t, P)], rhs=xTl_sb[:, :],
                start=False, stop=False,
            )
            nc.tensor.matmul(
                dot_ps[:, :], lhsT=yTl_sb[f][:, ts(t, P)], rhs=xTh_sb[:, :],
                start=False, stop=True,
            )
            return dot_ps

        def emit_exp(f, t, dot_ps):
            w_t = wpool.tile([P, IW], F32, name="w_t", tag="w")
            nc.scalar.activation(
                w_t[:, :], dot_ps[:, :], Act.Exp,
                bias=ysq_sb[f][:, t : t + 1], scale=1.0,
            )
            return w_t

        # software pipeline across both fields: dist runs DEPTH iterations
        # ahead of acc/s, exp runs in between, so PE and ACT never ping-pong.
        steps = [(f, t) for f in range(2) for t in range(NT)] * repeat
        DEPTH = 3
        dots = {}
        ws = {}
        accT_ps_f = {}
        s_ps_f = {}
        for f in range(2):
            accT_ps_f[f] = ps_acc.tile([P, IW], F32, name="accT_ps", tag="acc")
            s_ps_f[f] = ps_s.tile([1, IW], F32, name="s_ps", tag="s")
        for k in range(DEPTH):
            dots[k] = emit_dist(*steps[k])
            ws[k] = emit_exp(*steps[k], dots[k])
        accTr_ps = []

        def emit_field_epilogue(f):
            # accT -> SBUF -> per-chunk transpose back to [i, d]; s row -> SBUF.
            acc_sb = epi.tile([P, IW], F32, name="acc_sb", tag="accsb", bufs=2)
            nc.scalar.copy(acc_sb[:, :], accT_ps_f[f][:, :])
            accT_sb.append(acc_sb)
            nc.scalar.copy(srows[f][:, :], s_ps_f[f][:, :])
            tr = ps_acc.tile([P, NCH, P], F32, name="tr", tag="acc")
            for ch in range(NCH):
                nc.tensor.matmul(
                    tr[:, ch, :], lhsT=acc_sb[:, ts(ch, P)], rhs=ident[:, :],
                    is_transpose=True, start=(ch == 0), stop=(ch == NCH - 1),
                )
            accTr_ps.append(tr)

        passes = len(steps) // (2 * NT)
        for i, (f, t) in enumerate(steps):
            if i + DEPTH < len(steps):
                dots[i + DEPTH] = emit_dist(*steps[i + DEPTH])
                ws[i + DEPTH] = emit_exp(*steps[i + DEPTH], dots[i + DEPTH])
            w_t = ws.pop(i)
            dots.pop(i)
            nc.tensor.matmul(
                accT_ps_f[f][:, :], lhsT=y_sb[f][:, t, :], rhs=w_t[:, :],
                start=(t == 0), stop=(t == NT - 1),
            )
            nc.tensor.matmul(
                s_ps_f[f][:, :], lhsT=ones[:, :], rhs=w_t[:, :],
                start=(t == 0), stop=(t == NT - 1),
            )
            if t == NT - 1 and i >= len(steps) - 2 * NT:
                # last pass of this field: drain its accumulators now so the
                # copies/transposes overlap the other field's loop.
                emit_field_epilogue(f)

        # ---- epilogue ----
        # transpose s rows -> per-partition scalars sT[p, ch, f]
        sT_ps = ps_s.tile([P, NCH, 2], F32, name="sT_ps", tag="s")
        for k in range(2 * NCH):
            ch, f = divmod(k, 2)
            nc.tensor.matmul(
                sT_ps[:, ch, f : f + 1], lhsT=srows[f][0:1, ts(ch, P)],
                rhs=ident[0:1, 0:1],
                is_transpose=True, start=(k == 0), stop=(k == 2 * NCH - 1),
            )
        sT_sb = singles.tile([P, NCH, 2], F32, name="sT_sb", tag="sT_sb")
        nc.vector.tensor_copy(sT_sb[:, :, :], sT_ps[:, :, :])

        # scalar math on [P, NCH] tiles
        def small(tag):
            return singles.tile([P, NCH], F32, name=tag, tag=tag)

        rr = []          # r_f = 1/denom_f
        ratio = []       # ratio_f = s_f/denom_f
        for f in range(2):
            sraw = sT_sb[:, :, f]
            st = small(f"st{f}")
            nc.vector.tensor_mul(st[:, :], sraw, exb[:, :])          # exb*s_raw
            nc.vector.tensor_sub(st[:, :], st[:, :], wii[f][:, :])   # - w_ii
            dn = small(f"dn{f}")
            nc.vector.tensor_scalar_max(dn[:, :], st[:, :], EPS)
            r = small(f"r{f}")
            nc.vector.reciprocal(r[:, :], dn[:, :])
            ra = small(f"ra{f}")
            nc.vector.tensor_mul(ra[:, :], st[:, :], r[:, :])
            rr.append(r)
            ratio.append(ra)

        coefx = small("coefx")     # 0.5*ratio_n - ratio_p
        nc.vector.scalar_tensor_tensor(
            out=coefx[:, :], in0=ratio[1][:, :], scalar=0.5, in1=ratio[0][:, :],
            op0=Alu.mult, op1=Alu.subtract,
        )
        apscale = small("apscale")  # exb * r_p
        nc.vector.tensor_mul(apscale[:, :], exb[:, :], rr[0][:, :])
        anscale = small("anscale")  # -0.5 * exb * r_n
        nc.vector.scalar_tensor_tensor(
            out=anscale[:, :], in0=rr[1][:, :], scalar=-0.5, in1=exb[:, :],
            op0=Alu.mult, op1=Alu.mult,
        )
        pdscale = small("pdscale")  # -wii_p * r_p
        nc.vector.scalar_tensor_tensor(
            out=pdscale[:, :], in0=wii[0][:, :], scalar=-1.0, in1=rr[0][:, :],
            op0=Alu.mult, op1=Alu.mult,
        )
        ndscale = small("ndscale")  # +0.5 * wii_n * r_n
        nc.vector.scalar_tensor_tensor(
            out=ndscale[:, :], in0=wii[1][:, :], scalar=0.5, in1=rr[1][:, :],
            op0=Alu.mult, op1=Alu.mult,
        )

        # final combine per chunk:
        # v = x*coefx + accTr_p*apscale + accTr_n*anscale + ypd*pdscale + ynd*ndscale
        out_sb = singles.tile([P, NCH, D], F32, name="out_sb", tag="out_sb")
        for ch in range(NCH):
            ta = epi.tile([P, D], F32, name="ta", tag="ta")
            tb = epi.tile([P, D], F32, name="tb", tag="tb")
            nc.vector.tensor_scalar_mul(ta[:, :], x_sb[:, ch, :], coefx[:, ch : ch + 1])
            nc.vector.scalar_tensor_tensor(
                out=tb[:, :], in0=accTr_ps[0][:, ch, :], scalar=apscale[:, ch : ch + 1],
                in1=ta[:, :], op0=Alu.mult, op1=Alu.add,
            )
            ta2 = epi.tile([P, D], F32, name="ta2", tag="ta")
            nc.vector.scalar_tensor_tensor(
                out=ta2[:, :], in0=accTr_ps[1][:, ch, :], scalar=anscale[:, ch : ch + 1],
                in1=tb[:, :], op0=Alu.mult, op1=Alu.add,
            )
            tb2 = epi.tile([P, D], F32, name="tb2", tag="tb")
            nc.vector.scalar_tensor_tensor(
                out=tb2[:, :], in0=yd_sb[0][:, ch, :], scalar=pdscale[:, ch : ch + 1],
                in1=ta2[:, :], op0=Alu.mult, op1=Alu.add,
            )
            nc.vector.scalar_tensor_tensor(
                out=out_sb[:, ch, :], in0=yd_sb[1][:, ch, :], scalar=ndscale[:, ch : ch + 1],
                in1=tb2[:, :], op0=Alu.mult, op1=Alu.add,
            )

        nc.sync.dma_start(out_d[:, :].rearrange("(c p) d -> p c d", p=P), out_sb[:, :, :])

    return nc


def _split_multi_waits(nc):
    """The walrus build behind the PJRT path accepts at most ONE sync-wait per
    instruction (setupSyncWait 'Too many sync wait commands').  Hoist extra
    waits onto preceding same-engine NoOps, which each carry one wait."""
    from concourse import mybir

    for bb in nc.m.functions[0].blocks:
        out = []
        for inst in bb.instructions:
            si = inst.sync_info
            if (
                si is not None and si.on_wait and len(si.on_wait) > 1
                and type(inst).__name__ != "InstNoOp"
            ):
                waits = list(si.on_wait)
                for k, w in enumerate(waits[:-1]):
                    out.append(mybir.InstNoOp(
                        name=f"{inst.name}-wsplit{k}",
                        engine=inst.engine,
                        ins=[], outs=[],
                        sync_info=mybir.SyncInfo(on_wait=[w], on_update=[]),
                    ))
                si.on_wait = waits[-1:]
            out.append(inst)
        bb.instructions[:] = out
    return nc


def _get_nc(repeat=1):
    key = f"nc{repeat}"
    if key not in _CACHE:
        _CACHE[key] = _split_multi_waits(_build(repeat))
    return _CACHE[key]


def _get_raw_nc():
    """Unsplit build for CoreSim (which rejects wait-only NoOps)."""
    if "nc_raw" not in _CACHE:
        _CACHE["nc_raw"] = _build()
    return _CACHE["nc_raw"]


def _in_maps(x, y_pos, y_neg):
    xf = np.ascontiguousarray(np.asarray(x, dtype=np.float32).reshape(B, D))
    ypf = np.ascontiguousarray(np.asarray(y_pos, dtype=np.float32).reshape(B, D))
    ynf = np.ascontiguousarray(np.asarray(y_neg, dtype=np.float32).reshape(B, D))
    def _split16(aT):
        h = aT.astype(np.float16)
        l = (aT - h.astype(np.float32)).astype(np.float16)
        return np.ascontiguousarray(h), np.ascontiguousarray(l)

    ypTh, ypTl = _split16(ypf.T)
    ynTh, ynTl = _split16(ynf.T)

    def _ysqh(yf):
        h = (-0.5 * (yf.astype(np.float64) ** 2).sum(axis=1)).astype(np.float32)
        return np.ascontiguousarray(h.reshape(NT, P).T)

    ysqh_p = _ysqh(ypf)
    ysqh_n = _ysqh(ynf)
    maps = []
    for c in range(NCORES):
        sl = slice(c * IW, (c + 1) * IW)
        xTh, xTl = _split16(xf[sl].T)
        maps.append({
            "x": xf[sl],
            "xTh": xTh,
            "xTl": xTl,
            "y_pos": ypf,
            "y_neg": ynf,
            "yTh_pos": ypTh,
            "yTl_pos": ypTl,
            "yTh_neg": ynTh,
            "yTl_neg": ynTl,
            "yd_pos": ypf[sl],
            "yd_neg": ynf[sl],
            "ysqh_pos": ysqh_p,
            "ysqh_neg": ysqh_n,
        })
    return maps


def _run(in_maps, trace=False, **kw):
    from concourse.bass_utils import run_bass_kernel_spmd

    nc = _get_nc()
    return run_bass_kernel_spmd(nc, in_maps, list(range(NCORES)), trace=trace, **kw)


def kernel(x, y_pos, y_neg):
    res = _run(_in_maps(x, y_pos, y_neg))
    out = np.concatenate([res.results[c]["out"] for c in range(NCORES)], axis=0)
    return out.reshape(B, TA, DA).astype(np.float32)


# revision 19
# speedup vs baseline: 1.0290x; 1.0290x over previous
"""Trainium2 Bass kernel for nn_DriftingPolicy (Nadaraya-Watson RBF drift field).

Computes v = -drift(x, y_pos) + 0.5*drift(x, y_neg) where
  drift(x, y)_i = x_i * (s_i/denom_i) - (w @ y)_i / denom_i
  w_ij = exp(-||x_i - y_j||^2 / 2), diagonal (i==j) masked, s = rowsum(w),
  denom = max(s, 1e-8).

Sharding: rows of x (B=4096) split across 8 cores (512 rows each); y_pos/y_neg
replicated.  Per core, flash-style loop over 32 j-tiles of y:
  dist:  dot[j,i]  = sum_d y[j,d] x[i,d]        (PE, lhsT = y.T tile)
  w_raw = exp(dot - 0.5*||y_j||^2)              (ACT, per-partition bias)
  accT[d,i] += sum_j y[j,d] w_raw[j,i]          (PE, accumulating)
  s_raw[i]  += sum_j w_raw[j,i]                 (PE, ones-vector lhsT)
The per-i factor exp(-0.5*||x_i||^2) and the diagonal-mask correction
(subtract w_ii, computed directly from x and the core's diagonal y rows)
are folded into the epilogue scalars.  Host pre-transposes x and y so no
on-device data transposes are needed in the main loop.
"""

import numpy as np

B, TA, DA = 4096, 16, 8
D = TA * DA            # 128
NCORES = 8
IW = B // NCORES       # 512 query rows per core
P = 128                # partitions
NT = B // P            # 32 j-tiles
NCH = IW // P          # 4 i-chunks per core
EPS = 1e-8

_CACHE = {}


def _build(repeat=1):
    import concourse.bass as bass
    import concourse.tile as tile
    from concourse import mybir
    from concourse.masks import make_identity
    from concourse.bass import ts
    from contextlib import ExitStack

    F32 = mybir.dt.float32
    Alu = mybir.AluOpType
    Act = mybir.ActivationFunctionType

    nc = bass.Bass()
    x_d = nc.declare_dram_parameter("x", [IW, D], F32, isOutput=False)
    F16 = mybir.dt.float16
    xTh_d = nc.declare_dram_parameter("xTh", [D, IW], F16, isOutput=False)
    xTl_d = nc.declare_dram_parameter("xTl", [D, IW], F16, isOutput=False)
    yh_d = [
        nc.declare_dram_parameter("yh_pos", [B, D], F16, isOutput=False),
        nc.declare_dram_parameter("yh_neg", [B, D], F16, isOutput=False),
    ]
    yl_d = [
        nc.declare_dram_parameter("yl_pos", [B, D], F16, isOutput=False),
        nc.declare_dram_parameter("yl_neg", [B, D], F16, isOutput=False),
    ]
    al_d = nc.declare_dram_parameter("alpha2", [2, IW], F16, isOutput=False)
    yTh_d = [
        nc.declare_dram_parameter("yTh_pos", [D, B], F16, isOutput=False),
        nc.declare_dram_parameter("yTh_neg", [D, B], F16, isOutput=False),
    ]
    yTl_d = [
        nc.declare_dram_parameter("yTl_pos", [D, B], F16, isOutput=False),
        nc.declare_dram_parameter("yTl_neg", [D, B], F16, isOutput=False),
    ]
    yd_d = [
        nc.declare_dram_parameter("yd_pos", [IW, D], F32, isOutput=False),
        nc.declare_dram_parameter("yd_neg", [IW, D], F32, isOutput=False),
    ]
    ysq_d = [
        nc.declare_dram_parameter("ysqh_pos", [P, NT], F32, isOutput=False),
        nc.declare_dram_parameter("ysqh_neg", [P, NT], F32, isOutput=False),
    ]
    out_d = nc.declare_dram_parameter("out", [IW, D], F32, isOutput=True)

    with tile.TileContext(nc) as tc, ExitStack() as ctx:
        singles = ctx.enter_context(tc.tile_pool(name="singles", bufs=1))
        wpool = ctx.enter_context(tc.tile_pool(name="wpool", bufs=5))
        scrpool = ctx.enter_context(tc.tile_pool(name="scr", bufs=2))
        ps_dot = ctx.enter_context(tc.tile_pool(name="ps_dot", bufs=4, space="PSUM"))
        ps_acc = ctx.enter_context(tc.tile_pool(name="ps_acc", bufs=2, space="PSUM"))
        ps_s = ctx.enter_context(tc.tile_pool(name="ps_s", bufs=2, space="PSUM"))
        epi = ctx.enter_context(tc.tile_pool(name="epi", bufs=2))

        # ---- constants & inputs resident in SBUF ----
        ident = singles.tile([P, P], F32, name="ident", tag="ident")
        make_identity(nc, ident[:, :])
        ones16 = singles.tile([P, 1], F16, name="ones16", tag="ones16")
        nc.gpsimd.memset(ones16[:, :], 1.0)
        onesrow = singles.tile([2, P], F16, name="onesrow", tag="onesrow")
        nc.gpsimd.memset(onesrow[:, :], 1.0)
        alpha_sb = singles.tile([2, IW], F16, name="alpha_sb", tag="alpha_sb")

        HEAD = 4
        # tiles, allocated up front
        x_sb = singles.tile([P, NCH, D], F32, name="x_sb", tag="x_sb")
        xTh_sb = singles.tile([D, IW], F16, name="xTh_sb", tag="xTh_sb")
        xTl_sb = singles.tile([D, IW], F16, name="xTl_sb", tag="xTl_sb")
        yd_sb = [
            singles.tile([P, NCH, D], F32, name=f"yd{f}", tag=f"yd{f}")
            for f in range(2)
        ]
        yh_sb = [
            singles.tile([P, NT, D], F16, name=f"yh{f}", tag=f"yh{f}")
            for f in range(2)
        ]
        yl_sb = [
            singles.tile([P, NT, D], F16, name=f"yl{f}", tag=f"yl{f}")
            for f in range(2)
        ]
        yTh_sb = [
            singles.tile([D, B], F16, name=f"yTh{f}", tag=f"yTh{f}")
            for f in range(2)
        ]
        yTl_sb = [
            singles.tile([D, B], F16, name=f"yTl{f}", tag=f"yTl{f}")
            for f in range(2)
        ]
        ysq_sb = [
            singles.tile([P, NT], F32, name=f"ysq{f}", tag=f"ysq{f}")
            for f in range(2)
        ]
        yh_ap = [yh_d[f][:, :].rearrange("(t p) d -> p t d", p=P) for f in range(2)]
        yl_ap = [yl_d[f][:, :].rearrange("(t p) d -> p t d", p=P) for f in range(2)]
        # issue order == SP execution order: hot path (first tiles of field 0)
        # first, then bulk, then field 1, then epilogue-only data.
        nc.sync.dma_start(xTh_sb[:, :], xTh_d[:, :])
        nc.sync.dma_start(xTl_sb[:, :], xTl_d[:, :])
        nc.sync.dma_start(alpha_sb[:, :], al_d[:, :])
        nc.sync.dma_start(yTh_sb[0][:, 0 : HEAD * P], yTh_d[0][:, 0 : HEAD * P])
        nc.sync.dma_start(yTl_sb[0][:, 0 : HEAD * P], yTl_d[0][:, 0 : HEAD * P])
        nc.sync.dma_start(ysq_sb[0][:, :], ysq_d[0][:, :])
        nc.sync.dma_start(yh_sb[0][:, 0:HEAD, :], yh_ap[0][:, 0:HEAD, :])
        nc.sync.dma_start(yl_sb[0][:, 0:HEAD, :], yl_ap[0][:, 0:HEAD, :])
        nc.sync.dma_start(yTh_sb[0][:, HEAD * P : B], yTh_d[0][:, HEAD * P : B])
        nc.sync.dma_start(yTl_sb[0][:, HEAD * P : B], yTl_d[0][:, HEAD * P : B])
        nc.sync.dma_start(yh_sb[0][:, HEAD:NT, :], yh_ap[0][:, HEAD:NT, :])
        nc.sync.dma_start(yl_sb[0][:, HEAD:NT, :], yl_ap[0][:, HEAD:NT, :])
        nc.sync.dma_start(yTh_sb[1][:, :], yTh_d[1][:, :])
        nc.sync.dma_start(yTl_sb[1][:, :], yTl_d[1][:, :])
        nc.sync.dma_start(ysq_sb[1][:, :], ysq_d[1][:, :])
        nc.sync.dma_start(yh_sb[1][:, :, :], yh_ap[1][:, :, :])
        nc.sync.dma_start(yl_sb[1][:, :, :], yl_ap[1][:, :, :])
        nc.sync.dma_start(x_sb[:, :, :], x_d[:, :].rearrange("(c p) d -> p c d", p=P))
        for f in range(2):
            nc.sync.dma_start(
                yd_sb[f][:, :, :],
                yd_d[f][:, :].rearrange("(c p) d -> p c d", p=P),
            )

        # ---- per-row scalars: xsqh = -0.5*||x_i||^2, exb = exp(xsqh),
        #      wii_f = exp(-0.5*||x_i - ydiag_i||^2) ----

        wii = []
        for f in range(2):
            d2 = singles.tile([P, NCH], F32, name=f"d2_{f}", tag=f"d2_{f}")
            for ch in range(NCH):
                diff = scrpool.tile([P, D], F32, name="diff", tag="scr")
                nc.vector.tensor_sub(diff[:, :], x_sb[:, ch, :], yd_sb[f][:, ch, :])
                scr2 = scrpool.tile([P, D], F32, name="scr2", tag="scr")
                nc.vector.tensor_mul(scr2[:, :], diff[:, :], diff[:, :])
                nc.vector.reduce_sum(
                    d2[:, ch : ch + 1], scr2[:, :], axis=mybir.AxisListType.X
                )
            w = singles.tile([P, NCH], F32, name=f"wii{f}", tag=f"wii{f}")
            nc.scalar.activation(w[:, :], d2[:, :], Act.Exp, scale=-0.5)
            wii.append(w)

        # ---- main loop: two fields, 32 j-tiles each ----
        accT_sb = []   # [d, i] accumulators copied to SBUF
        srows = [
            singles.tile([1, IW], F32, name="srow0", tag="srow0"),
            singles.tile([1, IW], F32, name="srow1", tag="srow1"),
        ]
        def emit_dist(f, t):
            dot_ps = ps_dot.tile([P, IW], F32, name="dot_ps", tag="dot")
            # split-fp16 fp32 emulation: yh*xh + yh*xl + yl*xh  (ll term ~1e-6)
            nc.tensor.matmul(
                dot_ps[:, :], lhsT=yTh_sb[f][:, ts(t, P)], rhs=xTh_sb[:, :],
                start=True, stop=False,
            )
            nc.tensor.matmul(
                dot_ps[:, :], lhsT=yTh_sb[f][:, ts(t, P)], rhs=xTl_sb[:, :],
                start=False, stop=False,
            )
            nc.tensor.matmul(
                dot_ps[:, :], lhsT=yTl_sb[f][:, ts(t, P)], rhs=xTh_sb[:, :],
                start=False, stop=False,
            )
            nc.tensor.matmul(
                dot_ps[:, :], lhsT=onesrow[:, :], rhs=alpha_sb[:, :],
                start=False, stop=True,
            )
            return dot_ps

        def emit_exp(f, t, dot_ps):
            w_t = wpool.tile([P, IW], F32, name="w_t", tag="w")
            nc.scalar.activation(
                w_t[:, :], dot_ps[:, :], Act.Exp,
                bias=ysq_sb[f][:, t : t + 1], scale=1.0,
            )
            wh = wpool.tile([P, IW], F16, name="wh", tag="wh")
            nc.vector.tensor_copy(wh[:, :], w_t[:, :])
            wl = wpool.tile([P, IW], F16, name="wl", tag="wl")
            nc.vector.tensor_sub(wl[:, :], w_t[:, :], wh[:, :])
            return (wh, wl)

        # software pipeline across both fields: dist runs DEPTH iterations
        # ahead of acc/s, exp runs in between, so PE and ACT never ping-pong.
        steps = [(f, t) for f in range(2) for t in range(NT)] * repeat
        DEPTH = 3
        dots = {}
        ws = {}
        accT_ps_f = {}
        s_ps_f = {}
        for f in range(2):
            accT_ps_f[f] = ps_acc.tile([P, IW], F32, name="accT_ps", tag="acc")
            s_ps_f[f] = ps_s.tile([1, IW], F32, name="s_ps", tag="s")
        for k in range(DEPTH):
            dots[k] = emit_dist(*steps[k])
            ws[k] = emit_exp(*steps[k], dots[k])
        accTr_ps = []

        def emit_field_epilogue(f):
            # accT -> SBUF -> per-chunk transpose back to [i, d]; s row -> SBUF.
            acc_sb = epi.tile([P, IW], F32, name="acc_sb", tag="accsb", bufs=2)
            nc.scalar.copy(acc_sb[:, :], accT_ps_f[f][:, :])
            accT_sb.append(acc_sb)
            nc.scalar.copy(srows[f][:, :], s_ps_f[f][:, :])
            tr = ps_acc.tile([P, NCH, P], F32, name="tr", tag="acc")
            for ch in range(NCH):
                nc.tensor.matmul(
                    tr[:, ch, :], lhsT=acc_sb[:, ts(ch, P)], rhs=ident[:, :],
                    is_transpose=True, start=(ch == 0), stop=(ch == NCH - 1),
                )
            accTr_ps.append(tr)

        passes = len(steps) // (2 * NT)
        for i, (f, t) in enumerate(steps):
            if i + DEPTH < len(steps):
                dots[i + DEPTH] = emit_dist(*steps[i + DEPTH])
                ws[i + DEPTH] = emit_exp(*steps[i + DEPTH], dots[i + DEPTH])
            wh, wl = ws.pop(i)
            dots.pop(i)
            nc.tensor.matmul(
                accT_ps_f[f][:, :], lhsT=yh_sb[f][:, t, :], rhs=wh[:, :],
                start=(t == 0), stop=False,
            )
            nc.tensor.matmul(
                accT_ps_f[f][:, :], lhsT=yl_sb[f][:, t, :], rhs=wh[:, :],
                start=False, stop=False,
            )
            nc.tensor.matmul(
                accT_ps_f[f][:, :], lhsT=yh_sb[f][:, t, :], rhs=wl[:, :],
                start=False, stop=(t == NT - 1),
            )
            nc.tensor.matmul(
                s_ps_f[f][:, :], lhsT=ones16[:, :], rhs=wh[:, :],
                start=(t == 0), stop=False,
            )
            nc.tensor.matmul(
                s_ps_f[f][:, :], lhsT=ones16[:, :], rhs=wl[:, :],
                start=False, stop=(t == NT - 1),
            )
            if t == NT - 1 and i >= len(steps) - 2 * NT:
                # last pass of this field: drain its accumulators now so the
                # copies/transposes overlap the other field's loop.
                emit_field_epilogue(f)

        # ---- epilogue ----
        # transpose s rows -> per-partition scalars sT[p, ch, f]
        sT_ps = ps_s.tile([P, NCH, 2], F32, name="sT_ps", tag="s")
        for k in range(2 * NCH):
            ch, f = divmod(k, 2)
            nc.tensor.matmul(
                sT_ps[:, ch, f : f + 1], lhsT=srows[f][0:1, ts(ch, P)],
                rhs=ident[0:1, 0:1],
                is_transpose=True, start=(k == 0), stop=(k == 2 * NCH - 1),
            )
        sT_sb = singles.tile([P, NCH, 2], F32, name="sT_sb", tag="sT_sb")
        nc.vector.tensor_copy(sT_sb[:, :, :], sT_ps[:, :, :])

        # scalar math on [P, NCH] tiles
        def small(tag):
            return singles.tile([P, NCH], F32, name=tag, tag=tag)

        SCL = 2.0 ** -96   # w was computed scaled by 2^96 to fit fp16 range
        rr = []          # r_f = 1/denom_f
        ratio = []       # ratio_f = s_f/denom_f
        for f in range(2):
            sraw = sT_sb[:, :, f]
            st = small(f"st{f}")
            nc.vector.scalar_tensor_tensor(
                out=st[:, :], in0=sraw, scalar=SCL, in1=wii[f][:, :],
                op0=Alu.mult, op1=Alu.subtract,
            )
            dn = small(f"dn{f}")
            nc.vector.tensor_scalar_max(dn[:, :], st[:, :], EPS)
            r = small(f"r{f}")
            nc.vector.reciprocal(r[:, :], dn[:, :])
            ra = small(f"ra{f}")
            nc.vector.tensor_mul(ra[:, :], st[:, :], r[:, :])
            rr.append(r)
            ratio.append(ra)

        coefx = small("coefx")     # 0.5*ratio_n - ratio_p
        nc.vector.scalar_tensor_tensor(
            out=coefx[:, :], in0=ratio[1][:, :], scalar=0.5, in1=ratio[0][:, :],
            op0=Alu.mult, op1=Alu.subtract,
        )
        apscale = small("apscale")  # 2^-96 * r_p
        nc.vector.tensor_scalar_mul(apscale[:, :], rr[0][:, :], SCL)
        anscale = small("anscale")  # -0.5 * 2^-96 * r_n
        nc.vector.tensor_scalar_mul(anscale[:, :], rr[1][:, :], -0.5 * SCL)
        pdscale = small("pdscale")  # -wii_p * r_p
        nc.vector.scalar_tensor_tensor(
            out=pdscale[:, :], in0=wii[0][:, :], scalar=-1.0, in1=rr[0][:, :],
            op0=Alu.mult, op1=Alu.mult,
        )
        ndscale = small("ndscale")  # +0.5 * wii_n * r_n
        nc.vector.scalar_tensor_tensor(
            out=ndscale[:, :], in0=wii[1][:, :], scalar=0.5, in1=rr[1][:, :],
            op0=Alu.mult, op1=Alu.mult,
        )

        # final combine per chunk:
        # v = x*coefx + accTr_p*apscale + accTr_n*anscale + ypd*pdscale + ynd*ndscale
        out_sb = singles.tile([P, NCH, D], F32, name="out_sb", tag="out_sb")
        for ch in range(NCH):
            ta = epi.tile([P, D], F32, name="ta", tag="ta")
            tb = epi.tile([P, D], F32, name="tb", tag="tb")
            nc.vector.tensor_scalar_mul(ta[:, :], x_sb[:, ch, :], coefx[:, ch : ch + 1])
            nc.vector.scalar_tensor_tensor(
                out=tb[:, :], in0=accTr_ps[0][:, ch, :], scalar=apscale[:, ch : ch + 1],
                in1=ta[:, :], op0=Alu.mult, op1=Alu.add,
            )
            ta2 = epi.tile([P, D], F32, name="ta2", tag="ta")
            nc.vector.scalar_tensor_tensor(
                out=ta2[:, :], in0=accTr_ps[1][:, ch, :], scalar=anscale[:, ch : ch + 1],
                in1=tb[:, :], op0=Alu.mult, op1=Alu.add,
            )
            tb2 = epi.tile([P, D], F32, name="tb2", tag="tb")
            nc.vector.scalar_tensor_tensor(
                out=tb2[:, :], in0=yd_sb[0][:, ch, :], scalar=pdscale[:, ch : ch + 1],
                in1=ta2[:, :], op0=Alu.mult, op1=Alu.add,
            )
            nc.vector.scalar_tensor_tensor(
                out=out_sb[:, ch, :], in0=yd_sb[1][:, ch, :], scalar=ndscale[:, ch : ch + 1],
                in1=tb2[:, :], op0=Alu.mult, op1=Alu.add,
            )

        nc.sync.dma_start(out_d[:, :].rearrange("(c p) d -> p c d", p=P), out_sb[:, :, :])

    return nc


def _split_multi_waits(nc):
    """The walrus build behind the PJRT path accepts at most ONE sync-wait per
    instruction (setupSyncWait 'Too many sync wait commands').  Hoist extra
    waits onto preceding same-engine NoOps, which each carry one wait."""
    from concourse import mybir

    for bb in nc.m.functions[0].blocks:
        out = []
        for inst in bb.instructions:
            si = inst.sync_info
            if (
                si is not None and si.on_wait and len(si.on_wait) > 1
                and type(inst).__name__ != "InstNoOp"
            ):
                waits = list(si.on_wait)
                for k, w in enumerate(waits[:-1]):
                    out.append(mybir.InstNoOp(
                        name=f"{inst.name}-wsplit{k}",
                        engine=inst.engine,
                        ins=[], outs=[],
                        sync_info=mybir.SyncInfo(on_wait=[w], on_update=[]),
                    ))
                si.on_wait = waits[-1:]
            out.append(inst)
        bb.instructions[:] = out
    return nc


def _get_nc(repeat=1):
    key = f"nc{repeat}"
    if key not in _CACHE:
        _CACHE[key] = _split_multi_waits(_build(repeat))
    return _CACHE[key]


def _get_raw_nc():
    """Unsplit build for CoreSim (which rejects wait-only NoOps)."""
    if "nc_raw" not in _CACHE:
        _CACHE["nc_raw"] = _build()
    return _CACHE["nc_raw"]


def _in_maps(x, y_pos, y_neg):
    xf = np.ascontiguousarray(np.asarray(x, dtype=np.float32).reshape(B, D))
    ypf = np.ascontiguousarray(np.asarray(y_pos, dtype=np.float32).reshape(B, D))
    ynf = np.ascontiguousarray(np.asarray(y_neg, dtype=np.float32).reshape(B, D))
    def _split16(aT):
        h = aT.astype(np.float16)
        l = (aT - h.astype(np.float32)).astype(np.float16)
        return np.ascontiguousarray(h), np.ascontiguousarray(l)

    ypTh, ypTl = _split16(ypf.T)
    ynTh, ynTl = _split16(ynf.T)

    C96 = 96.0 * np.log(2.0)

    def _ysqh(yf):
        h = (-0.5 * (yf.astype(np.float64) ** 2).sum(axis=1) + C96).astype(np.float32)
        return np.ascontiguousarray(h.reshape(NT, P).T)

    ysqh_p = _ysqh(ypf)
    ysqh_n = _ysqh(ynf)
    yph, ypl = _split16(ypf)
    ynh, ynl = _split16(ynf)
    maps = []
    for c in range(NCORES):
        sl = slice(c * IW, (c + 1) * IW)
        xTh, xTl = _split16(xf[sl].T)
        alpha = (-0.5 * (xf[sl].astype(np.float64) ** 2).sum(axis=1))[None, :]
        ah, al = _split16(alpha.astype(np.float32))
        alpha2 = np.ascontiguousarray(np.concatenate([ah, al], axis=0))
        maps.append({
            "x": xf[sl],
            "xTh": xTh,
            "xTl": xTl,
            "alpha2": alpha2,
            "yh_pos": yph,
            "yl_pos": ypl,
            "yh_neg": ynh,
            "yl_neg": ynl,
            "yTh_pos": ypTh,
            "yTl_pos": ypTl,
            "yTh_neg": ynTh,
            "yTl_neg": ynTl,
            "yd_pos": ypf[sl],
            "yd_neg": ynf[sl],
            "ysqh_pos": ysqh_p,
            "ysqh_neg": ysqh_n,
        })
    return maps


def _run(in_maps, trace=False, **kw):
    from concourse.bass_utils import run_bass_kernel_spmd

    nc = _get_nc()
    return run_bass_kernel_spmd(nc, in_maps, list(range(NCORES)), trace=trace, **kw)


def kernel(x, y_pos, y_neg):
    res = _run(_in_maps(x, y_pos, y_neg))
    out = np.concatenate([res.results[c]["out"] for c in range(NCORES)], axis=0)
    return out.reshape(B, TA, DA).astype(np.float32)
